# revision 12
# baseline (speedup 1.0000x reference)
"""Fused attention-JVP kernel for TRN2, SPMD over 8 NeuronCores.

Sharding: pure data parallel over (B=2) x (M=4) = 8 cores; core c handles
batch b = c//4, tangent direction m = c%4. The primal path (qkv, softmax,
out, proj of x[b]) is recomputed on each core of a b-group (4x duplication
is cheaper than broadcasting the [H,S,S] softmax matrix).

All matmuls run in bf16 with fp32 PSUM accumulation. Softmax is computed
unnormalized (scores here are O(1), no max-subtraction needed):
  G   = exp(scores^T)            [k, q] per head
  Z   = G * scores_t^T
  OT  = [V | 1]^T @ G            rows 0..63 = unnorm out^T, row 64 = l
  OT2 = [V | 1]^T @ Z + [Vt|0]^T @ G    row 64 = u
  out^T   = OT[0:64] / l
  out_t^T = OT2[0:64] / l - (u/l) * out^T
then o = out @ W_proj^T + b_proj, o_t = out_t @ W_proj^T.
"""

import numpy as np

B, S, D, M, H = 2, 1024, 768, 4, 12
HD = D // H          # 64
KD = D // 128        # 6 contraction chunks
SC = S // 128        # 8 sequence chunks
SCALE = HD ** -0.5

_NC_CACHE = {}


def _build_nc():
    import concourse.mybir as mybir
    from concourse import bacc
    from concourse.tile import TileContext
    from concourse.bass import ts, ds
    from concourse.masks import make_identity

    f32 = mybir.dt.float32
    bf16 = mybir.dt.bfloat16
    EXP = mybir.ActivationFunctionType.Exp
    MUL = mybir.AluOpType.mult
    SUB = mybir.AluOpType.subtract
    ADD = mybir.AluOpType.add

    nc = bacc.Bacc("TRN2", target_bir_lowering=False, debug=False, num_devices=8)
    x_ext = nc.dram_tensor("x", [S, D], f32, kind="ExternalInput")
    xt_ext = nc.dram_tensor("xt", [S, D], f32, kind="ExternalInput")
    wqkv_ext = nc.dram_tensor("w_qkv", [3 * D, D], f32, kind="ExternalInput")
    wproj_ext = nc.dram_tensor("w_proj", [D, D], f32, kind="ExternalInput")
    bproj_ext = nc.dram_tensor("b_proj", [D], f32, kind="ExternalInput")
    out_ext = nc.dram_tensor("out", [2 * S, D], f32, kind="ExternalOutput")

    with TileContext(nc) as tc:
        with (
            tc.tile_pool(name="const", bufs=1) as const,
            tc.tile_pool(name="persist", bufs=1) as persist,
        ):
            ident = const.tile([128, 128], f32, tag="ident")
            make_identity(nc, ident)
            bias_bc = persist.tile([128, D], f32, tag="bias_bc")
            nc.sync.dma_start(
                bias_bc,
                bproj_ext.ap().rearrange("(o d) -> o d", o=1).to_broadcast((128, D)),
            )

            wprojT = persist.tile([128, KD, D], bf16, tag="wprojT")
            # qkT[:, j, :]  j<6: Q^T rows (pre-scaled by SCALE); j>=6: K^T rows
            qkT = persist.tile([128, 2 * KD, S], bf16, tag="qkT")
            qktT = persist.tile([128, 2 * KD, S], bf16, tag="qktT")
            # V augmented with a ones column per head (col HD) for the l/u sums
            vaug = persist.tile([128, SC, H, HD + 1], bf16, tag="vaug")
            vtaug = persist.tile([128, SC, H, HD + 1], bf16, tag="vtaug")
            outT = persist.tile([128, KD, S], bf16, tag="outT")
            outtT = persist.tile([128, KD, S], bf16, tag="outtT")

            nc.any.memset(vaug[:, :, :, HD : HD + 1], 1.0)
            nc.any.memset(vtaug[:, :, :, HD : HD + 1], 0.0)

            # ---------------- phase 0+1: load, transpose, project ----------
            with (
                tc.tile_pool(name="ph1", bufs=1) as ph1,
                tc.tile_pool(name="ld", bufs=3) as ld,
                tc.tile_pool(name="psA", bufs=4, space="PSUM") as psA,
                tc.tile_pool(name="ptr", bufs=2, space="PSUM") as ptr,
            ):
                wqkvT = ph1.tile([128, KD, 3 * D], bf16, tag="wqkvT")
                xT = ph1.tile([128, KD, S], bf16, tag="xT")
                xtT = ph1.tile([128, KD, S], bf16, tag="xtT")

                def load_T(dram_ap, dst, nrows_chunks, tag):
                    # dram [R, D] f32 -> dst [128, KD, R] bf16 (transposed)
                    for rc in range(nrows_chunks):
                        row = ld.tile([128, D], f32, tag="ldrow")
                        nc.sync.dma_start(row, dram_ap[ts(rc, 128), :])
                        for cb in range(KD):
                            pst = ptr.tile([128, 128], f32, tag="ptr")
                            nc.tensor.transpose(pst, row[:, ts(cb, 128)], ident)
                            nc.any.tensor_copy(dst[:, cb, ts(rc, 128)], pst)

                load_T(wqkv_ext.ap(), wqkvT, 3 * D // 128, "w")
                load_T(wproj_ext.ap(), wprojT, KD, "wp")
                load_T(x_ext.ap(), xT, SC, "x")
                load_T(xt_ext.ap(), xtT, SC, "xt")

                # Q^T/K^T (transposed output) for primal and tangent
                for src, dst in ((xT, qkT), (xtT, qktT)):
                    for j in range(2 * KD):  # rows j*128 of qkv^T (Q then K)
                        for qh in range(2):
                            ps = psA.tile([128, 512], f32, tag="psA")
                            for kd in range(KD):
                                nc.tensor.matmul(
                                    ps,
                                    lhsT=wqkvT[:, kd, ts(j, 128)],
                                    rhs=src[:, kd, ts(qh, 512)],
                                    start=(kd == 0),
                                    stop=(kd == KD - 1),
                                )
                            if j < KD:  # Q rows: fold in softmax scale
                                nc.any.tensor_scalar_mul(
                                    dst[:, j, ts(qh, 512)], ps, SCALE
                                )
                            else:
                                nc.any.tensor_copy(dst[:, j, ts(qh, 512)], ps)

                # V / Vt natural layout into the augmented tiles
                for src, dst in ((xT, vaug), (xtT, vtaug)):
                    for mc in range(SC):
                        for vh in range(2):  # 2 x 384 V columns
                            ps = psA.tile([128, 512], f32, tag="psA")
                            psv = ps[:, :384]
                            for kd in range(KD):
                                nc.tensor.matmul(
                                    psv,
                                    lhsT=src[:, kd, ts(mc, 128)],
                                    rhs=wqkvT[:, kd, ds(2 * D + vh * 384, 384)],
                                    start=(kd == 0),
                                    stop=(kd == KD - 1),
                                )
                            nc.any.tensor_copy(
                                dst[:, mc, ds(vh * 6, 6), 0:HD],
                                psv.rearrange("p (h e) -> p h e", e=HD),
                            )

            # ---------------- phase 2: attention per head-pair -------------
            with (
                tc.tile_pool(name="gz", bufs=4) as gz,
                tc.tile_pool(name="ep", bufs=2) as ep,
                tc.tile_pool(name="ps2", bufs=4, space="PSUM") as ps2p,
                tc.tile_pool(name="psacc", bufs=4, space="PSUM") as psaccp,
                tc.tile_pool(name="dr", bufs=4, space="DRAM") as dr,
            ):
                for hp in range(KD):  # head pair: heads 2hp (parts 0-63), 2hp+1 (64-127)
                    for qh in range(2):
                        Gs = [gz.tile([128, SC, 512], bf16, tag="G", name=f"G{i}") for i in range(2)]
                        Zs = [gz.tile([128, SC, 512], bf16, tag="Z", name=f"Z{i}") for i in range(2)]
                        # scores^T -> exp
                        for kc in range(SC):
                            pss = [ps2p.tile([128, 512], f32, tag="ps2", name=f"pss{i}") for i in range(2)]
                            for hh in range(2):
                                pb = hh * 64
                                nc.tensor.matmul(
                                    pss[hh],
                                    lhsT=qkT[pb : pb + 64, KD + hp, ts(kc, 128)],
                                    rhs=qkT[pb : pb + 64, hp, ts(qh, 512)],
                                    start=True,
                                    stop=True,
                                )
                            for hh in range(2):
                                nc.scalar.activation(Gs[hh][:, kc], pss[hh], EXP)
                        # scores_t^T -> Z = G * scores_t^T
                        for kc in range(SC):
                            pst = [ps2p.tile([128, 512], f32, tag="ps2", name=f"pst{i}") for i in range(2)]
                            for hh in range(2):
                                pb = hh * 64
                                nc.tensor.matmul(
                                    pst[hh],
                                    lhsT=qktT[pb : pb + 64, KD + hp, ts(kc, 128)],
                                    rhs=qkT[pb : pb + 64, hp, ts(qh, 512)],
                                    start=True,
                                    stop=False,
                                )
                                nc.tensor.matmul(
                                    pst[hh],
                                    lhsT=qkT[pb : pb + 64, KD + hp, ts(kc, 128)],
                                    rhs=qktT[pb : pb + 64, hp, ts(qh, 512)],
                                    start=False,
                                    stop=True,
                                )
                            for hh in range(2):
                                nc.any.tensor_tensor(
                                    Zs[hh][:, kc], pst[hh], Gs[hh][:, kc], MUL
                                )
                        # out / out_t accumulation + epilogue
                        for hh in range(2):
                            h = 2 * hp + hh
                            po = psaccp.tile([128, 512], f32, tag="psacc", name="po")[: HD + 1]
                            pt = psaccp.tile([128, 512], f32, tag="psacc", name="pt")[: HD + 1]
                            for kc in range(SC):
                                nc.tensor.matmul(
                                    po,
                                    lhsT=vaug[:, kc, h],
                                    rhs=Gs[hh][:, kc],
                                    start=(kc == 0),
                                    stop=(kc == SC - 1),
                                )
                            for kc in range(SC):
                                nc.tensor.matmul(
                                    pt,
                                    lhsT=vaug[:, kc, h],
                                    rhs=Zs[hh][:, kc],
                                    start=(kc == 0),
                                    stop=False,
                                )
                                nc.tensor.matmul(
                                    pt,
                                    lhsT=vtaug[:, kc, h],
                                    rhs=Gs[hh][:, kc],
                                    start=False,
                                    stop=(kc == SC - 1),
                                )
                            lcp = ep.tile([1, 512], f32, tag="lcp")
                            nc.any.tensor_copy(lcp, po[HD : HD + 1, :])
                            rinv = ep.tile([1, 512], f32, tag="rinv")
                            nc.vector.reciprocal_approx_fast(rinv, lcp)
                            rr = ep.tile([1, 512], f32, tag="rr")
                            nc.any.tensor_tensor(rr, pt[HD : HD + 1, :], rinv, MUL)
                            rid = dr.tile([1, 512], f32, name="rid")
                            nc.sync.dma_start(rid, rinv)
                            rrd = dr.tile([1, 512], f32, name="rrd")
                            nc.sync.dma_start(rrd, rr)
                            rbi = ep.tile([64, 512], f32, tag="rbi")
                            nc.sync.dma_start(rbi, rid.to_broadcast((64, 512)))
                            rbr = ep.tile([64, 512], f32, tag="rbr")
                            nc.sync.dma_start(rbr, rrd.to_broadcast((64, 512)))
                            pb = hh * 64
                            if hh == 0:
                                dstO = outT[0:64, hp, ts(qh, 512)]
                                dstOt = outtT[0:64, hp, ts(qh, 512)]
                            else:
                                dstO = ep.tile([64, 512], bf16, tag="stO")
                                dstOt = ep.tile([64, 512], bf16, tag="stOt")
                            nc.any.tensor_tensor(dstO, po[:HD, :], rbi, MUL)
                            tmp = ep.tile([64, 512], f32, tag="tmp")
                            nc.any.tensor_tensor(tmp, pt[:HD, :], rbi, MUL)
                            tmp2 = ep.tile([64, 512], f32, tag="tmp2")
                            nc.gpsimd.tensor_tensor(tmp2, dstO, rbr, MUL)
                            nc.gpsimd.tensor_tensor(dstOt, tmp, tmp2, SUB)
                            if hh == 1:
                                nc.sync.dma_start(
                                    outT[64:128, hp, ts(qh, 512)], dstO
                                )
                                nc.sync.dma_start(
                                    outtT[64:128, hp, ts(qh, 512)], dstOt
                                )

            # ---------------- phase 3: output projections ------------------
            with (
                tc.tile_pool(name="ow", bufs=4) as ow,
                tc.tile_pool(name="ps3", bufs=4, space="PSUM") as ps3p,
            ):
                for src, rbase, with_bias in ((outT, 0, True), (outtT, S, False)):
                    for mc in range(SC):
                        for nh in range(2):  # 2 x 384 output cols
                            ps = ps3p.tile([128, 512], f32, tag="ps3")
                            psv = ps[:, :384]
                            for kd in range(KD):
                                nc.tensor.matmul(
                                    psv,
                                    lhsT=src[:, kd, ts(mc, 128)],
                                    rhs=wprojT[:, kd, ds(nh * 384, 384)],
                                    start=(kd == 0),
                                    stop=(kd == KD - 1),
                                )
                            ob = ow.tile([128, 384], f32, tag="ob")
                            if with_bias:
                                nc.any.tensor_tensor(
                                    ob, psv, bias_bc[:, ds(nh * 384, 384)], ADD
                                )
                            else:
                                nc.any.tensor_copy(ob, psv)
                            nc.sync.dma_start(
                                out_ext.ap()[ds(rbase + mc * 128, 128), ds(nh * 384, 384)],
                                ob,
                            )

    nc.finalize()
    return nc


def _get_nc():
    if "nc" not in _NC_CACHE:
        _NC_CACHE["nc"] = _build_nc()
    return _NC_CACHE["nc"]


def kernel(x, x_tangent, W_qkv, W_proj, b_proj):
    import concourse.bass_utils as bu

    x = np.asarray(x, np.float32)
    x_tangent = np.asarray(x_tangent, np.float32)
    W_qkv = np.asarray(W_qkv, np.float32)
    W_proj = np.asarray(W_proj, np.float32)
    b_proj = np.asarray(b_proj, np.float32)

    nc = _get_nc()
    in_maps = []
    for c in range(8):
        b, m = divmod(c, 4)
        in_maps.append(
            {
                "x": np.ascontiguousarray(x[b]),
                "xt": np.ascontiguousarray(x_tangent[b, m]),
                "w_qkv": W_qkv,
                "w_proj": W_proj,
                "b_proj": b_proj,
            }
        )
    res = bu.run_bass_kernel_spmd(nc, in_maps, list(range(8))).results
    o = np.stack([res[0]["out"][:S], res[4]["out"][:S]]).astype(np.float32)
    o_t = np.stack(
        [
            np.stack([res[b * 4 + m]["out"][S:] for m in range(M)])
            for b in range(B)
        ]
    ).astype(np.float32)
    return o, o_t


# revision 13
# speedup vs baseline: 1.1259x; 1.1259x over previous
"""Fused attention-JVP kernel for TRN2, SPMD over 8 NeuronCores.

Sharding: pure data parallel over (B=2) x (M=4) = 8 cores; core c handles
batch b = c//4, tangent direction m = c%4. The primal path (qkv, softmax,
out, proj of x[b]) is recomputed on each core of a b-group (4x duplication
is cheaper than broadcasting the [H,S,S] softmax matrix).

All matmuls run in bf16 with fp32 PSUM accumulation. Softmax is computed
unnormalized (scores here are O(1), no max-subtraction needed):
  G   = exp(scores^T)            [k, q] per head
  Z   = G * scores_t^T
  OT  = [V | 1]^T @ G            rows 0..63 = unnorm out^T, row 64 = l
  OT2 = [V | 1]^T @ Z + [Vt|0]^T @ G    row 64 = u
  out^T   = OT[0:64] / l
  out_t^T = OT2[0:64] / l - (u/l) * out^T
then o = out @ W_proj^T + b_proj, o_t = out_t @ W_proj^T.
"""

import numpy as np

B, S, D, M, H = 2, 1024, 768, 4, 12
HD = D // H          # 64
KD = D // 128        # 6 contraction chunks
SC = S // 128        # 8 sequence chunks
SCALE = HD ** -0.5

_NC_CACHE = {}


def _build_nc():
    import concourse.mybir as mybir
    from concourse import bacc
    from concourse.tile import TileContext
    from concourse.bass import ts, ds
    from concourse.masks import make_identity

    f32 = mybir.dt.float32
    bf16 = mybir.dt.bfloat16
    EXP = mybir.ActivationFunctionType.Exp
    MUL = mybir.AluOpType.mult
    SUB = mybir.AluOpType.subtract
    ADD = mybir.AluOpType.add

    nc = bacc.Bacc("TRN2", target_bir_lowering=False, debug=False, num_devices=8)
    x_ext = nc.dram_tensor("x", [S, D], f32, kind="ExternalInput")
    xt_ext = nc.dram_tensor("xt", [S, D], f32, kind="ExternalInput")
    wqkv_ext = nc.dram_tensor("w_qkv", [3 * D, D], f32, kind="ExternalInput")
    wproj_ext = nc.dram_tensor("w_proj", [D, D], f32, kind="ExternalInput")
    bproj_ext = nc.dram_tensor("b_proj", [D], f32, kind="ExternalInput")
    out_ext = nc.dram_tensor("out", [2 * S, D], f32, kind="ExternalOutput")

    with TileContext(nc) as tc:
        with (
            tc.tile_pool(name="const", bufs=1) as const,
            tc.tile_pool(name="persist", bufs=1) as persist,
        ):
            ident = const.tile([128, 128], f32, tag="ident")
            make_identity(nc, ident)
            bias_bc = persist.tile([128, D], f32, tag="bias_bc")
            nc.sync.dma_start(
                bias_bc,
                bproj_ext.ap().rearrange("(o d) -> o d", o=1).to_broadcast((128, D)),
            )

            wprojT = persist.tile([128, KD, D], bf16, tag="wprojT")
            # qkT[:, j, :]  j<6: Q^T rows (pre-scaled by SCALE); j>=6: K^T rows
            qkT = persist.tile([128, 2 * KD, S], bf16, tag="qkT")
            qktT = persist.tile([128, 2 * KD, S], bf16, tag="qktT")
            # V augmented with a ones column per head (col HD) for the l/u sums
            vaug = persist.tile([128, SC, H, HD + 1], bf16, tag="vaug")
            vtaug = persist.tile([128, SC, H, HD + 1], bf16, tag="vtaug")
            outT = persist.tile([128, KD, S], bf16, tag="outT")
            outtT = persist.tile([128, KD, S], bf16, tag="outtT")

            nc.any.memset(vaug[:, :, :, HD : HD + 1], 1.0)
            nc.any.memset(vtaug[:, :, :, HD : HD + 1], 0.0)

            # ---------------- phase 0+1: load, transpose, project ----------
            with (
                tc.tile_pool(name="ph1", bufs=1) as ph1,
                tc.tile_pool(name="ld", bufs=3) as ld,
                tc.tile_pool(name="psA", bufs=4, space="PSUM") as psA,
                tc.tile_pool(name="ptr", bufs=2, space="PSUM") as ptr,
            ):
                wqkvT = ph1.tile([128, KD, 3 * D], bf16, tag="wqkvT")
                xT = ph1.tile([128, KD, S], bf16, tag="xT")
                xtT = ph1.tile([128, KD, S], bf16, tag="xtT")

                def load_T(dram_ap, dst, nrows_chunks, tag):
                    # dram [R, D] f32 -> dst [128, KD, R] bf16 (transposed)
                    for rc in range(nrows_chunks):
                        row = ld.tile([128, D], f32, tag="ldrow")
                        nc.sync.dma_start(row, dram_ap[ts(rc, 128), :])
                        for cb in range(KD):
                            pst = ptr.tile([128, 128], f32, tag="ptr")
                            nc.tensor.transpose(pst, row[:, ts(cb, 128)], ident)
                            nc.any.tensor_copy(dst[:, cb, ts(rc, 128)], pst)

                load_T(wqkv_ext.ap(), wqkvT, 3 * D // 128, "w")
                load_T(wproj_ext.ap(), wprojT, KD, "wp")
                load_T(x_ext.ap(), xT, SC, "x")
                load_T(xt_ext.ap(), xtT, SC, "xt")

                # Q^T/K^T (transposed output) for primal and tangent
                for src, dst in ((xT, qkT), (xtT, qktT)):
                    for j in range(2 * KD):  # rows j*128 of qkv^T (Q then K)
                        for qh in range(2):
                            ps = psA.tile([128, 512], f32, tag="psA")
                            for kd in range(KD):
                                nc.tensor.matmul(
                                    ps,
                                    lhsT=wqkvT[:, kd, ts(j, 128)],
                                    rhs=src[:, kd, ts(qh, 512)],
                                    start=(kd == 0),
                                    stop=(kd == KD - 1),
                                )
                            if j < KD:  # Q rows: fold in softmax scale
                                nc.any.tensor_scalar_mul(
                                    dst[:, j, ts(qh, 512)], ps, SCALE
                                )
                            else:
                                nc.any.tensor_copy(dst[:, j, ts(qh, 512)], ps)

                # V / Vt natural layout into the augmented tiles
                for src, dst in ((xT, vaug), (xtT, vtaug)):
                    for mc in range(SC):
                        for vh in range(2):  # 2 x 384 V columns
                            ps = psA.tile([128, 512], f32, tag="psA")
                            psv = ps[:, :384]
                            for kd in range(KD):
                                nc.tensor.matmul(
                                    psv,
                                    lhsT=src[:, kd, ts(mc, 128)],
                                    rhs=wqkvT[:, kd, ds(2 * D + vh * 384, 384)],
                                    start=(kd == 0),
                                    stop=(kd == KD - 1),
                                )
                            nc.any.tensor_copy(
                                dst[:, mc, ds(vh * 6, 6), 0:HD],
                                psv.rearrange("p (h e) -> p h e", e=HD),
                            )

            # ---------------- phase 2: attention per head-pair -------------
            with (
                tc.tile_pool(name="gz", bufs=4) as gz,
                tc.tile_pool(name="ep", bufs=2) as ep,
                tc.tile_pool(name="ps2", bufs=4, space="PSUM") as ps2p,
                tc.tile_pool(name="psacc", bufs=4, space="PSUM") as psaccp,
                tc.tile_pool(name="dr", bufs=4, space="DRAM") as dr,
            ):
                for hp in range(KD):  # head pair: heads 2hp (parts 0-63), 2hp+1 (64-127)
                    for qh in range(2):
                        Gs = [gz.tile([128, SC, 512], bf16, tag="G", name=f"G{i}") for i in range(2)]
                        Zs = [gz.tile([128, SC, 512], bf16, tag="Z", name=f"Z{i}") for i in range(2)]
                        # scores^T -> exp
                        for kc in range(SC):
                            pss = [ps2p.tile([128, 512], f32, tag="ps2", name=f"pss{i}") for i in range(2)]
                            for hh in range(2):
                                pb = hh * 64
                                nc.tensor.matmul(
                                    pss[hh],
                                    lhsT=qkT[pb : pb + 64, KD + hp, ts(kc, 128)],
                                    rhs=qkT[pb : pb + 64, hp, ts(qh, 512)],
                                    start=True,
                                    stop=True,
                                )
                            for hh in range(2):
                                nc.scalar.activation(Gs[hh][:, kc], pss[hh], EXP)
                        # scores_t^T -> Z = G * scores_t^T
                        for kc in range(SC):
                            pst = [ps2p.tile([128, 512], f32, tag="ps2", name=f"pst{i}") for i in range(2)]
                            for hh in range(2):
                                pb = hh * 64
                                nc.tensor.matmul(
                                    pst[hh],
                                    lhsT=qktT[pb : pb + 64, KD + hp, ts(kc, 128)],
                                    rhs=qkT[pb : pb + 64, hp, ts(qh, 512)],
                                    start=True,
                                    stop=False,
                                )
                                nc.tensor.matmul(
                                    pst[hh],
                                    lhsT=qkT[pb : pb + 64, KD + hp, ts(kc, 128)],
                                    rhs=qktT[pb : pb + 64, hp, ts(qh, 512)],
                                    start=False,
                                    stop=True,
                                )
                            for hh in range(2):
                                nc.any.tensor_tensor(
                                    Zs[hh][:, kc], pst[hh], Gs[hh][:, kc], MUL
                                )
                        # out / out_t accumulation + epilogue
                        for hh in range(2):
                            h = 2 * hp + hh
                            po = psaccp.tile([128, 512], f32, tag="psacc", name="po")[: HD + 1]
                            pt = psaccp.tile([128, 512], f32, tag="psacc", name="pt")[: HD + 1]
                            for kc in range(SC):
                                nc.tensor.matmul(
                                    po,
                                    lhsT=vaug[:, kc, h],
                                    rhs=Gs[hh][:, kc],
                                    start=(kc == 0),
                                    stop=(kc == SC - 1),
                                )
                            for kc in range(SC):
                                nc.tensor.matmul(
                                    pt,
                                    lhsT=vaug[:, kc, h],
                                    rhs=Zs[hh][:, kc],
                                    start=(kc == 0),
                                    stop=False,
                                )
                                nc.tensor.matmul(
                                    pt,
                                    lhsT=vtaug[:, kc, h],
                                    rhs=Gs[hh][:, kc],
                                    start=False,
                                    stop=(kc == SC - 1),
                                )
                            rinv = ep.tile([1, 512], f32, tag="rinv")
                            nc.vector.reciprocal(rinv, po[HD : HD + 1, :])
                            rr = ep.tile([1, 512], f32, tag="rr")
                            nc.any.tensor_tensor(rr, pt[HD : HD + 1, :], rinv, MUL)
                            rid = dr.tile([1, 512], f32, name="rid")
                            nc.sync.dma_start(rid, rinv)
                            rrd = dr.tile([1, 512], f32, name="rrd")
                            nc.sync.dma_start(rrd, rr)
                            rbi = ep.tile([64, 512], f32, tag="rbi")
                            nc.sync.dma_start(rbi, rid.to_broadcast((64, 512)))
                            rbr = ep.tile([64, 512], f32, tag="rbr")
                            nc.sync.dma_start(rbr, rrd.to_broadcast((64, 512)))
                            pb = hh * 64
                            if hh == 0:
                                dstO = outT[0:64, hp, ts(qh, 512)]
                                dstOt = outtT[0:64, hp, ts(qh, 512)]
                            else:
                                dstO = ep.tile([64, 512], bf16, tag="stO")
                                dstOt = ep.tile([64, 512], bf16, tag="stOt")
                            nc.any.tensor_tensor(dstO, po[:HD, :], rbi, MUL)
                            tmp = ep.tile([64, 512], f32, tag="tmp")
                            nc.any.tensor_tensor(tmp, pt[:HD, :], rbi, MUL)
                            tmp2 = ep.tile([64, 512], f32, tag="tmp2")
                            nc.gpsimd.tensor_tensor(tmp2, dstO, rbr, MUL)
                            nc.gpsimd.tensor_tensor(dstOt, tmp, tmp2, SUB)
                            if hh == 1:
                                nc.sync.dma_start(
                                    outT[64:128, hp, ts(qh, 512)], dstO
                                )
                                nc.sync.dma_start(
                                    outtT[64:128, hp, ts(qh, 512)], dstOt
                                )

            # ---------------- phase 3: output projections ------------------
            with (
                tc.tile_pool(name="ow", bufs=4) as ow,
                tc.tile_pool(name="ps3", bufs=4, space="PSUM") as ps3p,
            ):
                for src, rbase, with_bias in ((outT, 0, True), (outtT, S, False)):
                    for mc in range(SC):
                        for nh in range(2):  # 2 x 384 output cols
                            ps = ps3p.tile([128, 512], f32, tag="ps3")
                            psv = ps[:, :384]
                            for kd in range(KD):
                                nc.tensor.matmul(
                                    psv,
                                    lhsT=src[:, kd, ts(mc, 128)],
                                    rhs=wprojT[:, kd, ds(nh * 384, 384)],
                                    start=(kd == 0),
                                    stop=(kd == KD - 1),
                                )
                            ob = ow.tile([128, 384], f32, tag="ob")
                            if with_bias:
                                nc.any.tensor_tensor(
                                    ob, psv, bias_bc[:, ds(nh * 384, 384)], ADD
                                )
                            else:
                                nc.any.tensor_copy(ob, psv)
                            nc.sync.dma_start(
                                out_ext.ap()[ds(rbase + mc * 128, 128), ds(nh * 384, 384)],
                                ob,
                            )

    nc.finalize()
    return nc


def _get_nc():
    if "nc" not in _NC_CACHE:
        _NC_CACHE["nc"] = _build_nc()
    return _NC_CACHE["nc"]


def kernel(x, x_tangent, W_qkv, W_proj, b_proj):
    import concourse.bass_utils as bu

    x = np.asarray(x, np.float32)
    x_tangent = np.asarray(x_tangent, np.float32)
    W_qkv = np.asarray(W_qkv, np.float32)
    W_proj = np.asarray(W_proj, np.float32)
    b_proj = np.asarray(b_proj, np.float32)

    nc = _get_nc()
    in_maps = []
    for c in range(8):
        b, m = divmod(c, 4)
        in_maps.append(
            {
                "x": np.ascontiguousarray(x[b]),
                "xt": np.ascontiguousarray(x_tangent[b, m]),
                "w_qkv": W_qkv,
                "w_proj": W_proj,
                "b_proj": b_proj,
            }
        )
    res = bu.run_bass_kernel_spmd(nc, in_maps, list(range(8))).results
    o = np.stack([res[0]["out"][:S], res[4]["out"][:S]]).astype(np.float32)
    o_t = np.stack(
        [
            np.stack([res[b * 4 + m]["out"][S:] for m in range(M)])
            for b in range(B)
        ]
    ).astype(np.float32)
    return o, o_t
